# revision 8
# baseline (speedup 1.0000x reference)
"""EdgeEncoder kernel for Trainium2 (8 NeuronCores, row-sharded).

Reference (per pair (i, j) of an N x N grid):
    out[h, i, j] = (1/n_ij) * sum_l mask[i,j,l] * sum_d feats[idx[i,j,l], d] * W[l, h, d]
with n_ij = max(#valid l, 1), idx in [-1, E-1], -1 = padding.

Device strategy (per core, which owns 64 rows i):
  - Projected tables T_l[e, h] = sum_d feats[e,d] W[l,h,d] are built on PE as
    [128 channels, e] tiles: channel (16g+c): c<8 -> value column h=c, c>=8 ->
    "validity" column (constant 1 via an appended ones-feature row).  Row 0 of
    each l-block is zeros (padding target).  Operands are bf16 (fp32 PSUM
    accumulate; end-to-end rel err ~2.3e-3 vs the 2e-2 gate).  The features
    ship as 7 stripes of 17 partitions ([119, 1432]) so the load uses the
    wide DMA path (~2us instead of ~13us on 17 partitions) and the phase-0
    table is ready ~17us in.
  - gpsimd ap_gather: Q7 core g gathers the stream (pair in share_g) from the
    SBUF-resident table; all 16 channels of the core follow the stream, so
    values for all 8 heads AND the validity bit arrive in one pass.  One phase
    per l, table builds overlap the previous phase's gather.  ap_gather costs
    ~27.7ns/index/Q7-core and is the ~565us wall; the SWDGE dma_gather
    alternative measures ~9ns/index on only 2 Q7 cores and loses 2.6x.
  - DVE reduces over l into acc[(g,c), (i_l, j)]; counts land on channels c>=8.
  - The last phase runs in 4 quarter chunks with the finale fused per quarter
    (recip(max(count,1)), partition-shift DMA on the Scalar queue, multiply)
    so only ~1/4 trails the last gather; the output ships as 8 per-head DMAs
    (partition stride 16) to avoid serializing 32 small DMA dispatches.
"""

import numpy as np
import ml_dtypes

import concourse.bass as bass
import concourse.mybir as mybir
import concourse.tile as tile
from concourse import bacc
from concourse.bass_utils import run_bass_kernel_spmd

N, L, H, D, E = 512, 5, 8, 16, 10000
NCORES = 8
RPC = N // NCORES            # 64 rows (i) per core
IPG = RPC // 8               # 8 rows (i) per Q7 core / share
PAIRS_G = IPG * N            # 4096 pairs per share
BLK = E + 1                  # 10001 rows per l-block (row 0 = zeros)
QCH = 1024                   # pairs per gather chunk (last phase + finale)
STR = 3334                   # e-columns per feature stripe (3 stripes)
NSTR = 3                     # stripe k lives at partitions [32k, 32k+17)
SUB = 1024                   # e-cols per PSUM sub-tile (2 banks)
f32, i16 = mybir.dt.float32, mybir.dt.int16
bf16 = mybir.dt.bfloat16

IDXW_COLS = L * PAIRS_G // 16   # 1280 int16 cols per partition

_cached = {}


def build_nc():
    nc = bacc.Bacc()

    idxw_t = nc.dram_tensor("idxw", [128, IDXW_COLS], i16, kind="ExternalInput")
    fstr = nc.dram_tensor("fstr", [81, STR], bf16, kind="ExternalInput")
    wch = nc.dram_tensor("wch", [81, 5 * 128], bf16, kind="ExternalInput")
    out = nc.dram_tensor("out", [H, RPC, N], f32, kind="ExternalOutput")

    with tile.TileContext(nc) as tc:
        with (
            tc.tile_pool(name="const", bufs=1) as cpool,
            tc.tile_pool(name="tbl", bufs=2) as tpool,
            tc.tile_pool(name="mm", bufs=2, space="PSUM") as mmpool,
            tc.tile_pool(name="gth", bufs=2) as gpool,
            tc.tile_pool(name="acc", bufs=1) as apool,
        ):
            # tiny dummy gather: forces the gpsimd ucode library load to
            # happen here, overlapped with the input DMAs below
            zi = cpool.tile([128, 1], i16)
            nc.vector.memset(zi[:, :], 0)
            zt = cpool.tile([128, 16], f32)
            nc.vector.memset(zt[:, :], 0.0)
            zo = cpool.tile([128, 16], f32)
            nc.gpsimd.ap_gather(
                out_ap=zo[:, :], in_ap=zt[:, :], idxs_ap=zi[:, :],
                channels=128, num_elems=16, d=1, num_idxs=16,
            )

            wch_sb = cpool.tile([81, 5 * 128], bf16)
            nc.sync.dma_start(out=wch_sb[:, :], in_=wch[:, :])
            fstr_sb = cpool.tile([81, STR], bf16)
            nc.sync.dma_start(out=fstr_sb[:, :], in_=fstr[:, :])
            idxw = cpool.tile([128, IDXW_COLS], i16)
            nc.sync.dma_start(out=idxw[:, :], in_=idxw_t[:, :])

            acc = apool.tile([128, PAIRS_G], f32)       # [(g,c), (il, j)]
            rt = apool.tile([128, PAIRS_G], f32)        # finale scaled values
            tmp1 = apool.tile([128, QCH], f32)
            tmp2 = apool.tile([128, QCH], f32)
            tmp3 = apool.tile([128, QCH], f32)

            for l in range(L):
                tbl = tpool.tile([128, BLK], f32, tag="tbl")
                nc.vector.memset(tbl[:, 0:1], 0.0)
                for k in range(NSTR):
                    base = k * STR
                    ck = min(E - base, STR)
                    for s0 in range(0, ck, SUB):
                        cs = min(SUB, ck - s0)
                        ps = mmpool.tile([128, SUB], f32, space="PSUM", tag="mm")
                        for t0 in range(0, cs, 512):
                            cnt = min(512, cs - t0)
                            nc.tensor.matmul(
                                out=ps[:, t0:t0 + cnt],
                                lhsT=wch_sb[32 * k:32 * k + D + 1,
                                            l * 128:(l + 1) * 128],
                                rhs=fstr_sb[32 * k:32 * k + D + 1,
                                            s0 + t0:s0 + t0 + cnt],
                                start=True,
                                stop=True,
                            )
                        nc.vector.tensor_copy(
                            out=tbl[:, 1 + base + s0:1 + base + s0 + cs],
                            in_=ps[:, :cs],
                        )
                chunk = PAIRS_G if l < L - 1 else QCH
                for off in range(0, PAIRS_G, chunk):
                    gth = gpool.tile([128, PAIRS_G], f32, tag="gth")
                    c0 = l * (PAIRS_G // 16) + off // 16
                    nc.gpsimd.ap_gather(
                        out_ap=gth[:, 0:chunk],
                        in_ap=tbl[:, :],
                        idxs_ap=idxw[:, c0:c0 + chunk // 16],
                        channels=128,
                        num_elems=BLK,
                        d=1,
                        num_idxs=chunk,
                    )
                    asl = acc[:, off:off + chunk]
                    if l == 0:
                        nc.vector.tensor_copy(out=asl, in_=gth[:, 0:chunk])
                    else:
                        nc.vector.tensor_add(
                            out=asl, in0=asl, in1=gth[:, 0:chunk]
                        )
                    if l == L - 1:
                        # fused finale for this quarter: 1/max(count,1) on
                        # channels c>=8, shift -8 partitions (Scalar queue so
                        # it never queues behind output DMAs), scale values
                        sl = slice(off, off + QCH)
                        nc.vector.tensor_scalar_max(
                            out=tmp1[:, :], in0=acc[:, sl], scalar1=1.0
                        )
                        # ~2 ULP, ~2.8x faster than InstReciprocal
                        nc.vector.reciprocal_approx_accurate(
                            out=tmp2[:, :], in_=tmp1[:, :], scratch=tmp3[:, :]
                        )
                        nc.scalar.dma_start(
                            out=tmp3[0:120, :], in_=tmp2[8:128, :]
                        )
                        nc.vector.tensor_tensor(
                            out=rt[0:120, sl], in0=acc[0:120, sl],
                            in1=tmp3[0:120, :], op=mybir.AluOpType.mult,
                        )

            # one output DMA per head: partitions {16g+h}, j-contiguous runs
            rt3 = rt[:, :].rearrange("(g c) (i j) -> g c i j", c=16, j=N)
            for h in range(H):
                nc.sync.dma_start(
                    out=out[h, :, :].rearrange("(g i) j -> g i j", i=IPG),
                    in_=rt3[:, h, :, :],
                )
    nc.compile()
    return nc


def _host_prep(edge_features_s, edge_weights, shortest_path_edges):
    feats = np.asarray(edge_features_s, dtype=np.float32)
    ew = np.asarray(edge_weights, dtype=np.float32)
    spe = np.asarray(shortest_path_edges).astype(np.int64)

    # featsT17 [17, E]: feats^T with an appended ones row, shipped as 3
    # zero-padded stripes at partition bases 0/32/64 ([81, 3334])
    W = ew[1:L + 1].reshape(L, H, D)
    featsT17 = np.concatenate([feats.T, np.ones((1, E), np.float32)], axis=0)
    fstr = np.zeros((81, STR), np.float32)
    for k in range(NSTR):
        base = k * STR
        ck = min(E - base, STR)
        fstr[32 * k:32 * k + D + 1, :ck] = featsT17[:, base:base + ck]
    # W_CH [17, 5*128]: col l*128+p: p%16<8 -> (W[l, p%16, :], 0); else (0.., 1)
    # replicated at partition bases 0/32/64 to satisfy the PE base rule
    wch1 = np.zeros((D + 1, 5 * 128), np.float32)
    for l in range(L):
        for p in range(128):
            c = p % 16
            if c < H:
                wch1[:D, l * 128 + p] = W[l, c]
            else:
                wch1[D, l * 128 + p] = 1.0
    wch = np.zeros((81, 5 * 128), np.float32)
    for k in range(NSTR):
        wch[32 * k:32 * k + D + 1] = wch1
    fstr = np.ascontiguousarray(fstr.astype(ml_dtypes.bfloat16))
    wch = np.ascontiguousarray(wch.astype(ml_dtypes.bfloat16))

    comb = (spe + 1).astype(np.int32)   # [N, N, L], 0 = padding
    # per-device wrapped index streams
    idxw_all = np.zeros((NCORES, 128, IDXW_COLS), np.int16)
    for cdev in range(NCORES):
        sub = comb[cdev * RPC:(cdev + 1) * RPC]  # [64, 512, 5]
        for l in range(L):
            for g in range(8):
                flat = sub[g * IPG:(g + 1) * IPG][:, :, l].reshape(-1)
                wrapped = flat.reshape(PAIRS_G // 16, 16).T
                idxw_all[cdev, 16 * g:16 * g + 16,
                         l * (PAIRS_G // 16):(l + 1) * (PAIRS_G // 16)] = wrapped
    return fstr, wch, idxw_all


def kernel(edge_features_s, edge_weights, shortest_path_edges):
    if "nc" not in _cached:
        _cached["nc"] = build_nc()
    nc = _cached["nc"]

    fstr, wch, idxw_all = _host_prep(
        edge_features_s, edge_weights, shortest_path_edges
    )
    in_maps = []
    for c in range(NCORES):
        in_maps.append({
            "idxw": np.ascontiguousarray(idxw_all[c]),
            "fstr": fstr,
            "wch": wch,
        })
    res = run_bass_kernel_spmd(nc, in_maps, list(range(NCORES)))
    outs = [res.results[c]["out"].reshape(H, RPC, N) for c in range(NCORES)]
    return np.concatenate(outs, axis=1)
